# revision 1
# baseline (speedup 1.0000x reference)
"""Trainium2 Bass kernel for nn_DemLocGraphEncoder (4-layer GIN + variational heads).

Strategy
--------
The GIN segment-sum aggregation is recast as a dense matmul with a
host-precomputed (I + A)^T adjacency-multiplicity matrix (N=8192, so the
dense form maps perfectly onto the 128x128 TensorEngine; avg degree 32
makes gather/scatter paths no faster and far more complex).

Sharding: nodes are row-sharded 1024/core across 8 cores.  Each layer:
  1. AllGather node features x (node-major) -> x_full  [skipped for layer 0,
     whose input is replicated to every core]
  2. agg^T = x_full^T @ ATshard  on TensorE (feature-major output)
  3. MLP entirely in feature-major form: hT = relu(W1-matmuls + b1),
     xT = relu(W2-matmuls + b2)  (weights replicated, used directly as lhsT)
  4. PE-transpose xT -> node-major x_own, DMA to DRAM for the next AllGather.
Layer 3 folds w2_3 @ {wm,wv} into two fused [2048,128] heads (x3 is never
materialized), then z = mean + var*eps on the VectorEngine.

All matmuls run in fp16 (1 cycle/row on TRN2 PE, fp32 PSUM accumulation;
fp16 chosen over bf16 for its 10-bit mantissa; activations stay < ~1e3 so
no overflow).  Outputs are fp32.
"""

import sys

if "/opt/trn_rl_repo" not in sys.path:
    sys.path.insert(0, "/opt/trn_rl_repo")

import numpy as np

N, E, T, H, O, L = 8192, 262144, 256, 2048, 1024, 128
NC = 8
NS = N // NC          # 1024 nodes per core
P = 128
KT_NODES = N // P     # 64 k-tiles over source nodes
ND = NS // 512        # 2 free-dim tiles over own nodes

_PROGRAM_CACHE = {}


def _build_program(collectives=True, opts=None):
    opts = dict(opts or {})
    no_transpose = opts.get("no_transpose", False)   # sim-only: DMA instead of PE transpose
    drain_split = opts.get("drain_split", True)     # alternate agg drains DVE/ACT
    at_bufs = opts.get("at_bufs", 14)
    x_bufs = opts.get("x_bufs", 7)
    w_bufs = opts.get("w_bufs", 6)
    ps_bufs = opts.get("ps_bufs", 8)
    agg_group = opts.get("agg_group", 8)
    dma_tp = opts.get("dma_tp", False)     # feature-major AG + XBAR-transposed x loads
    l0_split = opts.get("l0_split", True)  # pipeline layer-0 n-halves with MLP
    import concourse.bass as bass  # noqa: F401
    import concourse.mybir as mybir
    import concourse.tile as tile
    from concourse import bacc
    from concourse.masks import make_identity

    f16 = mybir.dt.float16
    f32 = mybir.dt.float32
    AF = mybir.ActivationFunctionType

    nc = bacc.Bacc(
        "TRN2", target_bir_lowering=False, debug=False,
        num_devices=NC if collectives else 1,
    )

    # ---- I/O ----
    at_d = nc.dram_tensor("at_t", [KT_NODES, ND, P, 512], f16, kind="ExternalInput")
    x0_d = nc.dram_tensor("x0", [KT_NODES, P, T], f16, kind="ExternalInput")
    w_d = {}
    w_d["w1_0"] = nc.dram_tensor("w1_0", [H // P, P, T // P, P], f16, kind="ExternalInput")
    w_d["w2_0"] = nc.dram_tensor("w2_0", [H // P, P, H // P, P], f16, kind="ExternalInput")
    for l in (1, 2):
        w_d[f"w1_{l}"] = nc.dram_tensor(f"w1_{l}", [H // P, P, H // P, P], f16, kind="ExternalInput")
    w_d["w1_3"] = nc.dram_tensor("w1_3", [O // P, P, H // P, P], f16, kind="ExternalInput")
    for l in (1, 2):
        w_d[f"w2_{l}"] = nc.dram_tensor(f"w2_{l}", [H // P, P, H // P, P], f16, kind="ExternalInput")
    whm_d = nc.dram_tensor("whm", [P, O // P, P], f16, kind="ExternalInput")
    whv_d = nc.dram_tensor("whv", [P, O // P, P], f16, kind="ExternalInput")
    b_d = {}
    for l in range(3):
        b_d[f"b1_{l}"] = nc.dram_tensor(f"b1_{l}", [P, H // P], f32, kind="ExternalInput")
    b_d["b1_3"] = nc.dram_tensor("b1_3", [P, O // P], f32, kind="ExternalInput")
    for l in range(3):
        b_d[f"b2_{l}"] = nc.dram_tensor(f"b2_{l}", [P, H // P], f32, kind="ExternalInput")
    bhm_d = nc.dram_tensor("bhm", [P, 1], f32, kind="ExternalInput")
    bhv_d = nc.dram_tensor("bhv", [P, 1], f32, kind="ExternalInput")
    eps_d = nc.dram_tensor("epst", [P, NS], f32, kind="ExternalInput")

    z_d = nc.dram_tensor("zt", [P, NS], f32, kind="ExternalOutput")
    mean_d = nc.dram_tensor("meant", [P, NS], f32, kind="ExternalOutput")
    var_d = nc.dram_tensor("vart", [P, NS], f32, kind="ExternalOutput")

    HH = H // 2
    if dma_tp:
        # feature-major: xown [HH feats, NS nodes]; gathered [NC*HH, NS]
        xown = {(l, h): nc.dram_tensor(f"xown{l}_{h}", [HH, NS], f16)
                for l in (1, 2, 3) for h in (0, 1)}
        xg = {(l, h): nc.dram_tensor(f"xg{l}_{h}", [NC * HH, NS], f16, addr_space="Shared")
              for l in (1, 2, 3) for h in (0, 1)}
    else:
        xown = {(l, h): nc.dram_tensor(f"xown{l}_{h}", [NS, HH], f16)
                for l in (1, 2, 3) for h in (0, 1)}
        xg = {(l, h): nc.dram_tensor(f"xg{l}_{h}", [N, HH], f16, addr_space="Shared")
              for l in (1, 2, 3) for h in (0, 1)}

    rg = [list(range(NC))]

    with tile.TileContext(nc) as tc:
        with (
            tc.tile_pool(name="const", bufs=1) as const_p,
            tc.tile_pool(name="big", bufs=1) as big_p,
            tc.tile_pool(name="at", bufs=at_bufs) as at_p,
            tc.tile_pool(name="xslab", bufs=x_bufs) as x_p,
            tc.tile_pool(name="w", bufs=w_bufs) as w_p,
            tc.tile_pool(name="xo", bufs=2) as xo_p,
            tc.tile_pool(name="ps", bufs=ps_bufs, space="PSUM") as ps_p,
        ):
            ident = const_p.tile([P, P], f16, tag="ident")
            make_identity(nc, ident)

            bias_sb = {}
            for name, d in b_d.items():
                bias_sb[name] = const_p.tile(list(d.shape), f32, tag=f"b_{name}", name=f"b_{name}")
                nc.sync.dma_start(bias_sb[name][:], d[:])
            bhm_sb = const_p.tile([P, 1], f32, tag="bhm")
            nc.sync.dma_start(bhm_sb[:], bhm_d[:])
            bhv_sb = const_p.tile([P, 1], f32, tag="bhv")
            nc.sync.dma_start(bhv_sb[:], bhv_d[:])
            eps_sb = const_p.tile([P, NS], f32, tag="eps")
            nc.sync.dma_start(eps_sb[:], eps_d[:])
            whm_sb = const_p.tile([P, O // P, P], f16, tag="whm")
            nc.sync.dma_start(whm_sb[:], whm_d[:])
            whv_sb = const_p.tile([P, O // P, P], f16, tag="whv")
            nc.sync.dma_start(whv_sb[:], whv_d[:])

            def all_gather(l, h):
                if collectives:
                    nc.gpsimd.collective_compute(
                        "AllGather", mybir.AluOpType.bypass, replica_groups=rg,
                        ins=[xown[l, h][:].opt()], outs=[xg[l, h][:].opt()],
                    )
                else:
                    # sim-only stand-in: model the DMA traffic of the gather
                    for c in range(NC):
                        nc.sync.dma_start(xg[l, h][c * NS:(c + 1) * NS, :], xown[l, h][:])

            def agg(d_in, x_load_fn, uT, ns=None):
                """uT[:, mt, n*512:(n+1)*512] = sum_k x[k,m]^T @ AT[k,n]."""
                Mt = d_in // P
                for n in (range(ND) if ns is None else ns):
                    for g0 in range(0, Mt, agg_group):
                        gsz = min(agg_group, Mt - g0)
                        psums = [ps_p.tile([P, 512], f32, tag="mm", name=f"ps{_i}") for _i in range(gsz)]
                        for k in range(KT_NODES):
                            xs = x_p.tile([P, gsz * P], f16, tag="xslab")
                            x_load_fn(xs, k, g0 * P, gsz * P)
                            att = at_p.tile([P, 512], f16, tag="at")
                            nc.sync.dma_start(att[:], at_d[k, n])
                            for mi in range(gsz):
                                nc.tensor.matmul(
                                    psums[mi][:],
                                    lhsT=xs[:, mi * P:(mi + 1) * P],
                                    rhs=att[:],
                                    start=(k == 0),
                                    stop=(k == KT_NODES - 1),
                                )
                        for mi in range(gsz):
                            dst = uT[:, g0 + mi, n * 512:(n + 1) * 512]
                            if drain_split and mi % 2 == 1:
                                nc.scalar.copy(dst, psums[mi][:])
                            else:
                                nc.vector.tensor_copy(dst, psums[mi][:])

            def linear(w_dram, Kt, Mt, rhsT, outT, bias, relu, out_off=0, mts=None, ns=None):
                for mt in (range(Mt) if mts is None else mts):
                    ws = w_p.tile([P, Kt, P], f16, tag="w")
                    nc.sync.dma_start(ws[:], w_dram[mt])
                    for n in (range(ND) if ns is None else ns):
                        p = ps_p.tile([P, 512], f32, tag="mm")
                        for k in range(Kt):
                            nc.tensor.matmul(
                                p[:],
                                lhsT=ws[:, k, :],
                                rhs=rhsT[:, k, n * 512:(n + 1) * 512],
                                start=(k == 0),
                                stop=(k == Kt - 1),
                            )
                        nc.scalar.activation(
                            outT[:, out_off + mt, n * 512:(n + 1) * 512],
                            p[:],
                            AF.Relu if relu else AF.Identity,
                            bias=bias[:, mt:mt + 1],
                        )

            def transpose_store(xT, xown_dram, half):
                mt0 = half * (H // P // 2)
                nmt = H // P // 2
                if dma_tp:
                    # store feature-major directly; transposition happens on the
                    # post-AllGather XBAR load
                    for mt in range(nmt):
                        nc.sync.dma_start(
                            xown_dram[mt * P:(mt + 1) * P, :], xT[:, mt0 + mt, :]
                        )
                    return
                if no_transpose:
                    # sim-only: skip PE transposes, model DMA traffic directly
                    for j in range(NS // P):
                        for mt in range(nmt):
                            nc.sync.dma_start(
                                xown_dram[j * P:(j + 1) * P, mt * P:(mt + 1) * P],
                                xT[:, mt0 + mt, j * P:(j + 1) * P],
                            )
                    return
                for j in range(NS // P):
                    xo = xo_p.tile([P, nmt, P], f16, tag="xo")
                    for mt in range(nmt):
                        pt = ps_p.tile([P, P], f16, tag="mm")
                        nc.tensor.transpose(pt[:], xT[:, mt0 + mt, j * P:(j + 1) * P], ident[:])
                        if drain_split and mt % 2 == 1:
                            nc.scalar.copy(xo[:, mt, :], pt[:])
                        else:
                            nc.vector.tensor_copy(xo[:, mt, :], pt[:])
                    nc.sync.dma_start(xown_dram[j * P:(j + 1) * P, :], xo[:])

            uT0 = big_p.tile([P, T // P, NS], f16, tag="uT")
            hT = {}
            xT = {}

            # ---- layer 0 ----
            def x0_load(xs, k, c0, w):
                nc.sync.dma_start(xs[:], x0_d[k, :, c0:c0 + w])

            hT[0] = big_p.tile([P, H // P, NS], f16, tag="hT", name="hT0")
            xT[0] = big_p.tile([P, H // P, NS], f16, tag="xT", name="xT0")
            half0 = range(0, H // P // 2)
            half1 = range(H // P // 2, H // P)
            if not l0_split:
                with nc.named_scope("l0_agg"):
                    agg(T, x0_load, uT0)
                with nc.named_scope("l0_lin1"):
                    linear(w_d["w1_0"], T // P, H // P, uT0, hT[0], bias_sb["b1_0"], relu=True)
                for h, mts in ((0, half0), (1, half1)):
                    with nc.named_scope(f"l0_lin2_{h}"):
                        linear(w_d["w2_0"], H // P, H // P, hT[0], xT[0], bias_sb["b2_0"],
                               relu=True, mts=mts)
                    with nc.named_scope(f"l0_tp_{h}"):
                        transpose_store(xT[0], xown[1, h], h)
                    with nc.named_scope(f"ag1_{h}"):
                        all_gather(1, h)
            if l0_split:
              # layer 0's agg is AT-stream-bound (55us of MMs vs 90us of DMA), so
              # interleave its n-halves with MLP compute to cover the streaming
              with nc.named_scope("l0_agg0"):
                agg(T, x0_load, uT0, ns=[0])
              with nc.named_scope("l0_lin1_0"):
                  linear(w_d["w1_0"], T // P, H // P, uT0, hT[0], bias_sb["b1_0"],
                         relu=True, ns=[0])
              with nc.named_scope("l0_lin2_h0n0"):
                  linear(w_d["w2_0"], H // P, H // P, hT[0], xT[0], bias_sb["b2_0"],
                         relu=True, mts=half0, ns=[0])
              with nc.named_scope("l0_agg1"):
                  agg(T, x0_load, uT0, ns=[1])
              with nc.named_scope("l0_lin1_1"):
                  linear(w_d["w1_0"], T // P, H // P, uT0, hT[0], bias_sb["b1_0"],
                         relu=True, ns=[1])
              with nc.named_scope("l0_lin2_h0n1"):
                  linear(w_d["w2_0"], H // P, H // P, hT[0], xT[0], bias_sb["b2_0"],
                         relu=True, mts=half0, ns=[1])
              with nc.named_scope("l0_tp_0"):
                  transpose_store(xT[0], xown[1, 0], 0)
              with nc.named_scope("ag1_0"):
                  all_gather(1, 0)
              with nc.named_scope("l0_lin2_h1"):
                  linear(w_d["w2_0"], H // P, H // P, hT[0], xT[0], bias_sb["b2_0"],
                         relu=True, mts=half1)
              with nc.named_scope("l0_tp_1"):
                  transpose_store(xT[0], xown[1, 1], 1)
              with nc.named_scope("ag1_1"):
                  all_gather(1, 1)

            # ---- layers 1..3 ----
            for l in (1, 2, 3):
                uT = big_p.tile([P, H // P, NS], f16, tag="uT", name=f"uT{l}")
                with nc.named_scope(f"l{l}_agg"):
                    g0h, g1h = xg[l, 0], xg[l, 1]

                    def x_load(xs, k, c0, w, g0h=g0h, g1h=g1h):
                        gh = g0h if c0 < HH else g1h
                        c = c0 % HH
                        assert c + w <= HH
                        if dma_tp:
                            # xs[node, feat] <- XBAR-transposed [feat, node] block
                            r, j = k // (NS // P), k % (NS // P)
                            nc.sync.dma_start_transpose(
                                xs[:], gh[r * HH + c:r * HH + c + w, j * P:(j + 1) * P]
                            )
                        else:
                            nc.sync.dma_start(xs[:], gh[k * P:(k + 1) * P, c:c + w])

                    agg(H, x_load, uT)
                mt_out = (O if l == 3 else H) // P
                hT[l] = big_p.tile([P, mt_out, NS], f16, tag="hT", name=f"hTl{l}")
                with nc.named_scope(f"l{l}_lin1"):
                    linear(w_d[f"w1_{l}"], H // P, mt_out, uT, hT[l], bias_sb[f"b1_{l}"], relu=True)
                if l < 3:
                    xT[l] = big_p.tile([P, H // P, NS], f16, tag="xT", name=f"xTl{l}")
                    for h in (0, 1):
                        mts = range(h * (H // P // 2), (h + 1) * (H // P // 2))
                        with nc.named_scope(f"l{l}_lin2_{h}"):
                            linear(w_d[f"w2_{l}"], H // P, H // P, hT[l], xT[l],
                                   bias_sb[f"b2_{l}"], relu=True, mts=mts)
                        with nc.named_scope(f"l{l}_tp_{h}"):
                            transpose_store(xT[l], xown[l + 1, h], h)
                        with nc.named_scope(f"ag{l + 1}_{h}"):
                            all_gather(l + 1, h)

            # ---- fused heads ----
            mean_sb = const_p.tile([P, NS], f32, tag="mean_sb")
            var_sb = const_p.tile([P, NS], f32, tag="var_sb")
            z_sb = const_p.tile([P, NS], f32, tag="z_sb")
            with nc.named_scope("heads"):
                for W_sb, b_sb, o_sb in ((whm_sb, bhm_sb, mean_sb), (whv_sb, bhv_sb, var_sb)):
                    for n in range(ND):
                        p = ps_p.tile([P, 512], f32, tag="mm")
                        for k in range(O // P):
                            nc.tensor.matmul(
                                p[:],
                                lhsT=W_sb[:, k, :],
                                rhs=hT[3][:, k, n * 512:(n + 1) * 512],
                                start=(k == 0),
                                stop=(k == O // P - 1),
                            )
                        nc.scalar.activation(
                            o_sb[:, n * 512:(n + 1) * 512], p[:], AF.Identity,
                            bias=b_sb[:, 0:1],
                        )
                nc.vector.tensor_tensor(z_sb[:], var_sb[:], eps_sb[:], mybir.AluOpType.mult)
                nc.vector.tensor_tensor(z_sb[:], z_sb[:], mean_sb[:], mybir.AluOpType.add)
                nc.sync.dma_start(mean_d[:], mean_sb[:])
                nc.sync.dma_start(var_d[:], var_sb[:])
                nc.sync.dma_start(z_d[:], z_sb[:])

    nc.compile()
    return nc


def _tile_lhsT(w):
    """[K, M] fp16 -> [Mt, 128, Kt, 128]; slab [mt] is SBUF-ready [128p, Kt, 128m]."""
    K, M = w.shape
    Kt, Mt = K // P, M // P
    return np.ascontiguousarray(w.reshape(Kt, P, Mt, P).transpose(2, 1, 0, 3))


def _bias_t(b):
    """[M] fp32 -> [128, Mt] (partition = feature within tile)."""
    return np.ascontiguousarray(b.reshape(-1, P).T).astype(np.float32)


def prepare_inputs(inputs):
    """Host-side preprocessing: adjacency build + layout tiling. Returns in_maps."""
    f16 = np.float16
    eeg_nodes = np.asarray(inputs["eeg_nodes"], np.float32)
    eeg_idx = np.asarray(inputs["eeg_idx"])
    src = eeg_idx[0].astype(np.int64)
    dst = eeg_idx[1].astype(np.int64)

    counts = np.bincount(src * N + dst, minlength=N * N).reshape(N, N)
    AT = counts.astype(np.float32)
    AT[np.arange(N), np.arange(N)] += 1.0  # fold GIN's (1+eps)*x self-term, eps=0
    AT16 = AT.astype(f16)
    del AT, counts

    # Activations explode to ~1.3e5 by layer 3 (> fp16 max).  Since relu is
    # positively homogeneous, scale each of layers 0-2's output by S=1/16
    # (exact power of 2), folded into w2/b2; heads unscale via x S^-3.
    S = np.float32(1.0 / 16.0)
    c = [np.float32(1.0), S, S * S, S * S * S]  # cumulative scale of x_l input

    common = {}
    common["x0"] = np.ascontiguousarray(eeg_nodes.astype(f16).reshape(KT_NODES, P, T))
    for l in range(4):
        common[f"w1_{l}"] = _tile_lhsT(np.asarray(inputs[f"w1_{l}"], np.float32).astype(f16))
        common[f"b1_{l}"] = _bias_t(np.asarray(inputs[f"b1_{l}"], np.float32) * c[l])
    for l in range(3):
        common[f"w2_{l}"] = _tile_lhsT((np.asarray(inputs[f"w2_{l}"], np.float32) * S).astype(f16))
        common[f"b2_{l}"] = _bias_t(np.asarray(inputs[f"b2_{l}"], np.float32) * c[l + 1])

    # fused heads:  mean = h3 @ (w2_3 @ wm) + (b2_3 @ wm + bm); h3 arrives
    # scaled by c[3] so the fused weight is unscaled by 1/c[3].
    w2_3 = np.asarray(inputs["w2_3"], np.float32)
    b2_3 = np.asarray(inputs["b2_3"], np.float32)
    wm = np.asarray(inputs["wm"], np.float32)
    wv = np.asarray(inputs["wv"], np.float32)
    W2m = ((w2_3 @ wm) / c[3]).astype(f16)
    W2v = ((w2_3 @ wv) / c[3]).astype(f16)
    common["whm"] = _tile_lhsT(W2m)[0]
    common["whv"] = _tile_lhsT(W2v)[0]
    common["bhm"] = (b2_3 @ wm + np.asarray(inputs["bm"], np.float32)).reshape(P, 1).astype(np.float32)
    common["bhv"] = (b2_3 @ wv + np.asarray(inputs["bv"], np.float32)).reshape(P, 1).astype(np.float32)

    eps = np.asarray(inputs["eps"], np.float32)
    in_maps = []
    for c in range(NC):
        m = dict(common)
        blk = AT16[:, c * NS:(c + 1) * NS]
        m["at_t"] = np.ascontiguousarray(
            blk.reshape(KT_NODES, P, ND, 512).transpose(0, 2, 1, 3)
        )
        m["epst"] = np.ascontiguousarray(eps[c * NS:(c + 1) * NS, :].T)
        in_maps.append(m)
    return in_maps


def get_program():
    if "nc" not in _PROGRAM_CACHE:
        _PROGRAM_CACHE["nc"] = _build_program()
    return _PROGRAM_CACHE["nc"]


def assemble_outputs(results):
    z = np.empty((N, L), np.float32)
    mean = np.empty((N, L), np.float32)
    var = np.empty((N, L), np.float32)
    for c in range(NC):
        z[c * NS:(c + 1) * NS] = results[c]["zt"].T
        mean[c * NS:(c + 1) * NS] = results[c]["meant"].T
        var[c * NS:(c + 1) * NS] = results[c]["vart"].T
    return z, mean, var


def kernel(**inputs):
    from concourse.bass_utils import run_bass_kernel_spmd

    nc = get_program()
    in_maps = prepare_inputs(inputs)
    res = run_bass_kernel_spmd(nc, in_maps, core_ids=list(range(NC)))
    return assemble_outputs(res.results)



# revision 2
# speedup vs baseline: 1.3009x; 1.3009x over previous
"""Trainium2 Bass kernel for nn_DemLocGraphEncoder (4-layer GIN + variational heads).

Strategy (v2 — fp8 DoubleRow aggregation, transpose-free MLP)
-------------------------------------------------------------
The GIN segment-sum is a dense matmul against a host-built (I + A)^T
adjacency-count matrix.  Nodes are row-sharded 1024/core across 8 cores.

Aggregation runs in fp8-e4m3 with PE DoubleRow double-pumping: both the
adjacency counts (small integers — exact in fp8) and the gathered node
features (quantized; measured end-to-end rel err ~5.6e-3, budget 2e-2) are
fp8, so each matmul contracts 256 source nodes (2 k-tiles) at once.
Feature-major psum output [128 feat, 512 dst].

MLP stays fp16: lin1 keeps weights stationary (feature-major hT out);
lin2 instead makes the *h node-chunks* stationary and streams K-major w2,
producing node-major output directly — no PE transposes at all.  The b2
bias enters via an all-ones [128,128] matmul against b2/128 replicated.
lin2 drains ACT-relu straight to fp8, stored in a pair-major layout
([ft, kp_l, i, o, f]) so next-layer lhsT slabs are single contiguous
128KB DMA reads after the fp8 AllGather (1MB/rank per half).

Layer 3 folds w2_3 @ {wm,wv} into two [2048->1024->128] fused heads
(x3 never materialized), then z = mean + var*eps on the VectorEngine.

Activations scaled by S=1/16 per layer (folded into w2/b2, exact power
of 2) to keep fp8/fp16 ranges safe; heads unscale by S^-3.
"""

import sys

if "/opt/trn_rl_repo" not in sys.path:
    sys.path.insert(0, "/opt/trn_rl_repo")

import numpy as np
import ml_dtypes

N, E, T, H, O, L = 8192, 262144, 256, 2048, 1024, 128
NC = 8
NS = N // NC          # 1024 nodes per core
P = 128
KP = N // 256         # 32 k-pair tiles over source nodes (256 nodes each)
ND = NS // 512        # 2 free-dim tiles over own dst nodes
HH = H // 2           # 1024 feats per AllGather half

_PROGRAM_CACHE = {}

E4NP = ml_dtypes.float8_e4m3


def _build_program(collectives=True, opts=None):
    opts = dict(opts or {})
    w2q_bufs = opts.get("w2q_bufs", 5)
    xs_bufs = opts.get("xs_bufs", 8)
    at_bufs = opts.get("at_bufs", 8)
    w1_bufs = opts.get("w1_bufs", 3)
    ps_bufs = opts.get("ps_bufs", 8)
    import concourse.bass as bass  # noqa: F401
    import concourse.mybir as mybir
    import concourse.tile as tile
    from concourse import bacc

    f8 = mybir.dt.float8e4
    f16 = mybir.dt.float16
    f32 = mybir.dt.float32
    AF = mybir.ActivationFunctionType
    DR = mybir.MatmulPerfMode.DoubleRow

    nc = bacc.Bacc(
        "TRN2", target_bir_lowering=False, debug=False,
        num_devices=NC if collectives else 1,
    )

    # ---- I/O ----
    # adjacency (I + A)^T, own dst columns, fp8, pair-major per n-tile
    at_d = nc.dram_tensor("at8", [ND, KP, P, 2, 512], f8, kind="ExternalInput")
    # layer-0 features, fp8, pair-major
    x0_d = nc.dram_tensor("x08", [KP, P, 2, T], f8, kind="ExternalInput")
    w1_d = {}
    w1_d[0] = nc.dram_tensor("w1_0", [H // P, P, T // P, P], f16, kind="ExternalInput")
    for l in (1, 2):
        w1_d[l] = nc.dram_tensor(f"w1_{l}", [H // P, P, H // P, P], f16, kind="ExternalInput")
    w1_d[3] = nc.dram_tensor("w1_3", [O // P, P, H // P, P], f16, kind="ExternalInput")
    w2_d = {}
    for l in (0, 1, 2):
        w2_d[l] = nc.dram_tensor(f"w2k_{l}", [P, H // P, H], f16, kind="ExternalInput")
    b1_d = {}
    for l in range(3):
        b1_d[l] = nc.dram_tensor(f"b1_{l}", [P, H // P], f32, kind="ExternalInput")
    b1_d[3] = nc.dram_tensor("b1_3", [P, O // P], f32, kind="ExternalInput")
    b2r_d = {}
    for l in (0, 1, 2):
        b2r_d[l] = nc.dram_tensor(f"b2r_{l}", [P, H], f16, kind="ExternalInput")
    whm_d = nc.dram_tensor("whm", [P, O // P, P], f16, kind="ExternalInput")
    whv_d = nc.dram_tensor("whv", [P, O // P, P], f16, kind="ExternalInput")
    bhm_d = nc.dram_tensor("bhm", [P, 1], f32, kind="ExternalInput")
    bhv_d = nc.dram_tensor("bhv", [P, 1], f32, kind="ExternalInput")
    eps_d = nc.dram_tensor("epst", [P, NS], f32, kind="ExternalInput")

    z_d = nc.dram_tensor("zt", [P, NS], f32, kind="ExternalOutput")
    mean_d = nc.dram_tensor("meant", [P, NS], f32, kind="ExternalOutput")
    var_d = nc.dram_tensor("vart", [P, NS], f32, kind="ExternalOutput")

    # per-layer gathered features, fp8 pair-major: [ft, kp_l, i, o, f]
    xown = {(l, h): nc.dram_tensor(f"xown{l}_{h}", [2, NS // 256, P, 2, 512], f8)
            for l in (1, 2, 3) for h in (0, 1)}
    xg = {(l, h): nc.dram_tensor(f"xg{l}_{h}", [2 * NC, NS // 256, P, 2, 512], f8,
                                 addr_space="Shared")
          for l in (1, 2, 3) for h in (0, 1)}

    rg = [list(range(NC))]

    with tile.TileContext(nc) as tc:
        with (
            tc.tile_pool(name="const", bufs=1) as const_p,
            tc.tile_pool(name="big", bufs=1) as big_p,
            tc.tile_pool(name="at", bufs=at_bufs) as at_p,
            tc.tile_pool(name="xs", bufs=xs_bufs) as x_p,
            tc.tile_pool(name="w1", bufs=w1_bufs) as w1_p,
            tc.tile_pool(name="w2", bufs=w2q_bufs) as w2_p,
            tc.tile_pool(name="b2", bufs=2) as b2_p,
            tc.tile_pool(name="xo", bufs=4) as xo_p,
            tc.tile_pool(name="stg", bufs=6) as stg_p,
            tc.tile_pool(name="ps", bufs=ps_bufs, space="PSUM") as ps_p,
        ):
            ones_sb = const_p.tile([P, P], f16, tag="ones")
            nc.any.memset(ones_sb[:], 1.0)

            b1_sb = {}
            for l, d in b1_d.items():
                b1_sb[l] = const_p.tile(list(d.shape), f32, tag=f"b1_{l}", name=f"b1_{l}")
                nc.sync.dma_start(b1_sb[l][:], d[:])
            bhm_sb = const_p.tile([P, 1], f32, tag="bhm")
            nc.sync.dma_start(bhm_sb[:], bhm_d[:])
            bhv_sb = const_p.tile([P, 1], f32, tag="bhv")
            nc.sync.dma_start(bhv_sb[:], bhv_d[:])
            eps_sb = const_p.tile([P, NS], f32, tag="eps")
            nc.sync.dma_start(eps_sb[:], eps_d[:])
            whm_sb = const_p.tile([P, O // P, P], f16, tag="whm")
            nc.sync.dma_start(whm_sb[:], whm_d[:])
            whv_sb = const_p.tile([P, O // P, P], f16, tag="whv")
            nc.sync.dma_start(whv_sb[:], whv_d[:])

            def all_gather(l, h):
                if collectives:
                    nc.gpsimd.collective_compute(
                        "AllGather", mybir.AluOpType.bypass, replica_groups=rg,
                        ins=[xown[l, h][:].opt()], outs=[xg[l, h][:].opt()],
                    )
                else:
                    for c in range(NC):
                        nc.sync.dma_start(xg[l, h][c * 2:(c + 1) * 2], xown[l, h][:])

            drain_ctr = [0]

            def drain(dst, psum):
                if drain_ctr[0] % 2 == 1:
                    nc.scalar.copy(dst, psum[:])
                else:
                    nc.vector.tensor_copy(dst, psum[:])
                drain_ctr[0] += 1

            def agg(l, d_in, x_load):
                """uT[:, mt, n*512:(n+1)*512] += fp8 DoubleRow over 32 k-pairs."""
                Mt = d_in // P
                GW = min(4, Mt)
                uT = big_p.tile([P, Mt, NS], f16, tag="uT", name=f"uT{l}")
                for n in range(ND):
                    for g in range(Mt // GW):
                        psums = [ps_p.tile([P, 512], f32, tag="mm", name=f"agps{mi}")
                                 for mi in range(GW)]
                        for kp in range(KP):
                            xs = x_p.tile([P, 2, GW * P], f8, tag="xs")
                            x_load(xs, kp, g)
                            att = at_p.tile([P, 2, 512], f8, tag="at")
                            nc.sync.dma_start(att[:], at_d[n, kp])
                            for mi in range(GW):
                                nc.tensor.matmul(
                                    psums[mi][:],
                                    lhsT=xs[:, :, mi * P:(mi + 1) * P],
                                    rhs=att[:],
                                    start=(kp == 0),
                                    stop=(kp == KP - 1),
                                    perf_mode=DR,
                                )
                        for mi in range(GW):
                            drain(uT[:, g * GW + mi, n * 512:(n + 1) * 512], psums[mi])
                return uT

            def lin1(l, uT, Kt, Mt):
                hT = big_p.tile([P, Mt, NS], f16, tag="hT", name=f"hT{l}")
                for mt in range(Mt):
                    ws = w1_p.tile([P, Kt, P], f16, tag="w1")
                    nc.sync.dma_start(ws[:], w1_d[l][mt])
                    for n in range(ND):
                        p = ps_p.tile([P, 512], f32, tag="mm")
                        for k in range(Kt):
                            nc.tensor.matmul(
                                p[:],
                                lhsT=ws[:, k, :],
                                rhs=uT[:, k, n * 512:(n + 1) * 512],
                                start=(k == 0),
                                stop=(k == Kt - 1),
                            )
                        nc.scalar.activation(
                            hT[:, mt, n * 512:(n + 1) * 512], p[:],
                            AF.Relu, bias=b1_sb[l][:, mt:mt + 1],
                        )
                return hT

            def lin2(l, hT):
                """x_{l+1} = relu(h @ w2 + b2) node-major, fp8, -> xown + AG."""
                Kt = H // P
                b2rep = b2_p.tile([P, H], f16, tag="b2rep")
                nc.sync.dma_start(b2rep[:], b2r_d[l][:])
                for h in (0, 1):
                    w2q = []
                    for q in range(4):
                        wq = w2_p.tile([P, 4, HH], f16, tag="w2q", name=f"w2q{q}")
                        nc.sync.dma_start(
                            wq[:], w2_d[l][:, q * 4:(q + 1) * 4, h * HH:(h + 1) * HH])
                        w2q.append(wq)
                    for jb in range(NS // 256):
                        psums = [[ps_p.tile([P, 512], f32, tag="mm", name=f"l2ps{o}{ft}")
                                  for ft in range(2)] for o in range(2)]
                        for o in range(2):
                            for ft in range(2):
                                nc.tensor.matmul(
                                    psums[o][ft][:],
                                    lhsT=ones_sb[:],
                                    rhs=b2rep[:, h * HH + ft * 512:h * HH + (ft + 1) * 512],
                                    start=True, stop=False,
                                )
                        for kt in range(Kt):
                            wq = w2q[kt // 4]
                            for o in range(2):
                                j = jb * 2 + o
                                for ft in range(2):
                                    nc.tensor.matmul(
                                        psums[o][ft][:],
                                        lhsT=hT[:, kt, j * P:(j + 1) * P],
                                        rhs=wq[:, kt % 4, ft * 512:(ft + 1) * 512],
                                        start=False, stop=(kt == Kt - 1),
                                    )
                        for ft in range(2):
                            xo = xo_p.tile([P, 2, 512], f8, tag="xo")
                            for o in range(2):
                                nc.scalar.activation(xo[:, o, :], psums[o][ft][:], AF.Relu)
                            nc.sync.dma_start(xown[l + 1, h][ft, jb], xo[:])
                    all_gather(l + 1, h)

            # ---- layer 0 ----
            def x0_load(xs, kp, g):
                nc.sync.dma_start(xs[:], x0_d[kp])

            with nc.named_scope("l0_agg"):
                uT0 = agg(0, T, x0_load)
            with nc.named_scope("l0_lin1"):
                hT0 = lin1(0, uT0, T // P, H // P)
            with nc.named_scope("l0_lin2"):
                lin2(0, hT0)

            # ---- layers 1..3 ----
            hT3 = None
            for l in (1, 2, 3):
                g0 = xg[l, 0]
                g1 = xg[l, 1]

                def x_load(xs, kp, g, g0=g0, g1=g1):
                    h, fg = divmod(g, 2)
                    gh = g0 if h == 0 else g1
                    r, kpl = divmod(kp, 4)
                    nc.sync.dma_start(xs[:], gh[r * 2 + fg, kpl])

                with nc.named_scope(f"l{l}_agg"):
                    uT = agg(l, H, x_load)
                mt_out = (O if l == 3 else H) // P
                with nc.named_scope(f"l{l}_lin1"):
                    hT = lin1(l, uT, H // P, mt_out)
                if l < 3:
                    with nc.named_scope(f"l{l}_lin2"):
                        lin2(l, hT)
                else:
                    hT3 = hT

            # ---- fused heads ----
            with nc.named_scope("heads"):
                for n in range(ND):
                    nsl = slice(n * 512, (n + 1) * 512)
                    stages = {}
                    for key, W_sb, b_sb in (("m", whm_sb, bhm_sb), ("v", whv_sb, bhv_sb)):
                        p = ps_p.tile([P, 512], f32, tag="mm")
                        for k in range(O // P):
                            nc.tensor.matmul(
                                p[:],
                                lhsT=W_sb[:, k, :],
                                rhs=hT3[:, k, nsl],
                                start=(k == 0),
                                stop=(k == O // P - 1),
                            )
                        st = stg_p.tile([P, 512], f32, tag="stg", name=f"st{key}")
                        nc.scalar.activation(st[:], p[:], AF.Identity, bias=b_sb[:, 0:1])
                        stages[key] = st
                    stz = stg_p.tile([P, 512], f32, tag="stg", name="stz")
                    nc.vector.tensor_tensor(
                        stz[:], stages["v"][:], eps_sb[:, nsl], mybir.AluOpType.mult)
                    nc.vector.tensor_tensor(
                        stz[:], stz[:], stages["m"][:], mybir.AluOpType.add)
                    nc.sync.dma_start(mean_d[:, nsl], stages["m"][:])
                    nc.sync.dma_start(var_d[:, nsl], stages["v"][:])
                    nc.sync.dma_start(z_d[:, nsl], stz[:])

    nc.compile()
    return nc


def _tile_lhsT(w):
    """[K, M] fp16 -> [Mt, 128, Kt, 128]; slab [mt] is SBUF-ready [128p, Kt, 128m]."""
    K, M = w.shape
    Kt, Mt = K // P, M // P
    return np.ascontiguousarray(w.reshape(Kt, P, Mt, P).transpose(2, 1, 0, 3))


def _bias_t(b):
    """[M] fp32 -> [128, Mt] (partition = feature within tile)."""
    return np.ascontiguousarray(b.reshape(-1, P).T).astype(np.float32)


def _q8(x):
    return np.clip(x, -240.0, 240.0).astype(E4NP)


def prepare_inputs(inputs):
    """Host-side preprocessing: adjacency build + layout tiling. Returns in_maps."""
    f16 = np.float16
    eeg_nodes = np.asarray(inputs["eeg_nodes"], np.float32)
    eeg_idx = np.asarray(inputs["eeg_idx"])
    src = eeg_idx[0].astype(np.int64)
    dst = eeg_idx[1].astype(np.int64)

    counts = np.bincount(src * N + dst, minlength=N * N).reshape(N, N)
    AT = counts.astype(np.float32)
    AT[np.arange(N), np.arange(N)] += 1.0  # fold GIN's (1+eps)*x self-term, eps=0
    AT8 = _q8(AT)  # counts <= ~5: exact in fp8
    del AT, counts

    # Scale layers 0-2 outputs by S=1/16 (exact power of 2, folded into w2/b2)
    # to keep activations in fp8/fp16 range; heads unscale via x S^-3.
    S = np.float32(1.0 / 16.0)
    c = [np.float32(1.0), S, S * S, S * S * S]  # cumulative scale of x_l input

    common = {}
    # x0 pair-major fp8: [kp, i, o, t] = x0[kp*256 + o*128 + i, t]
    common["x08"] = np.ascontiguousarray(
        _q8(eeg_nodes).reshape(KP, 2, P, T).transpose(0, 2, 1, 3))
    for l in range(4):
        common[f"w1_{l}"] = _tile_lhsT(np.asarray(inputs[f"w1_{l}"], np.float32).astype(f16))
        common[f"b1_{l}"] = _bias_t(np.asarray(inputs[f"b1_{l}"], np.float32) * c[l])
    for l in range(3):
        w2s = (np.asarray(inputs[f"w2_{l}"], np.float32) * S).astype(f16)
        # K-major: [i, kt, m] = w2[kt*128 + i, m]
        common[f"w2k_{l}"] = np.ascontiguousarray(
            w2s.reshape(H // P, P, H).transpose(1, 0, 2))
        b2s = (np.asarray(inputs[f"b2_{l}"], np.float32) * c[l + 1]) / np.float32(P)
        common[f"b2r_{l}"] = np.ascontiguousarray(
            np.broadcast_to(b2s.astype(f16), (P, H)))

    # fused heads: mean = h3 @ (w2_3 @ wm) + (b2_3 @ wm + bm); h3 carries c[3]
    w2_3 = np.asarray(inputs["w2_3"], np.float32)
    b2_3 = np.asarray(inputs["b2_3"], np.float32)
    wm = np.asarray(inputs["wm"], np.float32)
    wv = np.asarray(inputs["wv"], np.float32)
    W2m = ((w2_3 @ wm) / c[3]).astype(f16)
    W2v = ((w2_3 @ wv) / c[3]).astype(f16)
    common["whm"] = _tile_lhsT(W2m)[0]
    common["whv"] = _tile_lhsT(W2v)[0]
    common["bhm"] = (b2_3 @ wm + np.asarray(inputs["bm"], np.float32)).reshape(P, 1).astype(np.float32)
    common["bhv"] = (b2_3 @ wv + np.asarray(inputs["bv"], np.float32)).reshape(P, 1).astype(np.float32)

    eps = np.asarray(inputs["eps"], np.float32)
    in_maps = []
    for cc in range(NC):
        m = dict(common)
        blk = AT8[:, cc * NS:(cc + 1) * NS].astype(np.float32)
        # [n, kp, i, o, f] = AT[kp*256 + o*128 + i, n*512 + f]
        m["at8"] = np.ascontiguousarray(
            _q8(blk.reshape(KP, 2, P, ND, 512).transpose(3, 0, 2, 1, 4)))
        m["epst"] = np.ascontiguousarray(eps[cc * NS:(cc + 1) * NS, :].T)
        in_maps.append(m)
    return in_maps


def get_program():
    if "nc" not in _PROGRAM_CACHE:
        _PROGRAM_CACHE["nc"] = _build_program()
    return _PROGRAM_CACHE["nc"]


def assemble_outputs(results):
    z = np.empty((N, L), np.float32)
    mean = np.empty((N, L), np.float32)
    var = np.empty((N, L), np.float32)
    for c in range(NC):
        z[c * NS:(c + 1) * NS] = results[c]["zt"].T
        mean[c * NS:(c + 1) * NS] = results[c]["meant"].T
        var[c * NS:(c + 1) * NS] = results[c]["vart"].T
    return z, mean, var


def kernel(**inputs):
    from concourse.bass_utils import run_bass_kernel_spmd

    nc = get_program()
    in_maps = prepare_inputs(inputs)
    res = run_bass_kernel_spmd(nc, in_maps, core_ids=list(range(NC)))
    return assemble_outputs(res.results)


# revision 3
# speedup vs baseline: 1.7105x; 1.3148x over previous
"""Trainium2 Bass kernel for nn_DemLocGraphEncoder (4-layer GIN + variational heads).

Strategy (v3 — fp8 DoubleRow aggregation, transpose-free MLP, batched DMA)
--------------------------------------------------------------------------
The GIN segment-sum is a dense matmul against a host-built (I + A)^T
adjacency-count matrix.  Nodes are row-sharded 1024/core across 8 cores.

Aggregation runs in fp8-e4m3 with PE DoubleRow double-pumping: adjacency
counts (small integers — exact in fp8) and gathered node features
(quantized; measured end-to-end rel err ~5.9e-3, budget 2e-2) are both
fp8, so each matmul contracts 256 source nodes at once.  Feature-major
psum output [128 feat, 512 dst].  Gathered x lives in a pair-major,
feature-contiguous layout ([i, kp_l, o, ft, f]) so aggregation lhsT
slabs load as single 1MB DMAs with 8KB/partition lines (>=75% of HBM
peak; 128KB/1KB-line loads measured ~40% and starved the PE in v2).
AT streams as 512KB blocks on the second HWDGE ring (nc.scalar).

MLP stays fp16: lin1 keeps weights stationary (feature-major hT out);
lin2 makes the *h node-chunks* stationary and streams K-major w2,
producing node-major output directly — no PE transposes.  b2 enters via
an all-ones [128,128] matmul against b2/128 replicated.  lin2 drains
ACT-relu straight to fp8 into xown, AllGathered per half (1MB/rank).

Layer 3 folds w2_3 @ {wm,wv} into two fused [2048->128] heads (x3 never
materialized), then z = mean + var*eps on the VectorEngine.

Activations scaled by S=1/16 per layer (folded into w2/b2, exact power
of 2) to keep fp8/fp16 ranges safe; heads unscale by S^-3.
"""

import sys

if "/opt/trn_rl_repo" not in sys.path:
    sys.path.insert(0, "/opt/trn_rl_repo")

import numpy as np
import ml_dtypes

N, E, T, H, O, L = 8192, 262144, 256, 2048, 1024, 128
NC = 8
NS = N // NC          # 1024 nodes per core
P = 128
KP = N // 256         # 32 k-pair tiles over source nodes (256 nodes each)
NR = 8                # r-blocks of 4 k-pairs (1024 source nodes each)
ND = NS // 512        # 2 free-dim tiles over own dst nodes
HH = H // 2           # 1024 feats per AllGather half

_PROGRAM_CACHE = {}

E4NP = ml_dtypes.float8_e4m3


def _build_program(collectives=True, opts=None):
    opts = dict(opts or {})
    w2q_bufs = opts.get("w2q_bufs", 4)
    xs_bufs = opts.get("xs_bufs", 3)
    at_bufs = opts.get("at_bufs", 3)
    w1_bufs = opts.get("w1_bufs", 3)
    ps_bufs = opts.get("ps_bufs", 8)
    import concourse.bass as bass  # noqa: F401
    import concourse.mybir as mybir
    import concourse.tile as tile
    from concourse import bacc

    f8 = mybir.dt.float8e4
    f16 = mybir.dt.float16
    f32 = mybir.dt.float32
    AF = mybir.ActivationFunctionType
    DR = mybir.MatmulPerfMode.DoubleRow

    nc = bacc.Bacc(
        "TRN2", target_bir_lowering=False, debug=False,
        num_devices=NC if collectives else 1,
    )

    # ---- I/O ----
    # adjacency (I + A)^T fp8: [n, r, i, kp_l, o, f]
    at_d = nc.dram_tensor("at8", [ND, NR, P, 4, 2, 512], f8, kind="ExternalInput")
    # layer-0 features fp8: [rb, i, kpj, o, t]  (rb = 8 k-pairs)
    x0_d = nc.dram_tensor("x08", [4, P, 8, 2, T], f8, kind="ExternalInput")
    w1_d = {}
    w1_d[0] = nc.dram_tensor("w1_0", [H // P, P, T // P, P], f16, kind="ExternalInput")
    for l in (1, 2):
        w1_d[l] = nc.dram_tensor(f"w1_{l}", [H // P, P, H // P, P], f16, kind="ExternalInput")
    w1_d[3] = nc.dram_tensor("w1_3", [O // P, P, H // P, P], f16, kind="ExternalInput")
    w2_d = {}
    for l in (0, 1, 2):
        w2_d[l] = nc.dram_tensor(f"w2k_{l}", [P, H // P, H], f16, kind="ExternalInput")
    b1_d = {}
    for l in range(3):
        b1_d[l] = nc.dram_tensor(f"b1_{l}", [P, H // P], f32, kind="ExternalInput")
    b1_d[3] = nc.dram_tensor("b1_3", [P, O // P], f32, kind="ExternalInput")
    b2r_d = {}
    for l in (0, 1, 2):
        b2r_d[l] = nc.dram_tensor(f"b2r_{l}", [P, H], f16, kind="ExternalInput")
    whm_d = nc.dram_tensor("whm", [P, O // P, P], f16, kind="ExternalInput")
    whv_d = nc.dram_tensor("whv", [P, O // P, P], f16, kind="ExternalInput")
    bhm_d = nc.dram_tensor("bhm", [P, 1], f32, kind="ExternalInput")
    bhv_d = nc.dram_tensor("bhv", [P, 1], f32, kind="ExternalInput")
    eps_d = nc.dram_tensor("epst", [P, NS], f32, kind="ExternalInput")

    z_d = nc.dram_tensor("zt", [P, NS], f32, kind="ExternalOutput")
    mean_d = nc.dram_tensor("meant", [P, NS], f32, kind="ExternalOutput")
    var_d = nc.dram_tensor("vart", [P, NS], f32, kind="ExternalOutput")

    # per-layer gathered features fp8, pair-major feature-contiguous:
    # xown[i, kp_l, o, ft, f];  xg concat on i across ranks -> [r*128+i, ...]
    xown = {(l, h): nc.dram_tensor(f"xown{l}_{h}", [P, NS // 256, 2, 2, 512], f8)
            for l in (1, 2, 3) for h in (0, 1)}
    xg = {(l, h): nc.dram_tensor(f"xg{l}_{h}", [NC * P, NS // 256, 2, 2, 512], f8,
                                 addr_space="Shared")
          for l in (1, 2, 3) for h in (0, 1)}

    rg = [list(range(NC))]

    with tile.TileContext(nc) as tc:
        with (
            tc.tile_pool(name="const", bufs=1) as const_p,
            tc.tile_pool(name="big", bufs=1) as big_p,
            tc.tile_pool(name="at", bufs=at_bufs) as at_p,
            tc.tile_pool(name="xs", bufs=xs_bufs) as x_p,
            tc.tile_pool(name="w1", bufs=w1_bufs) as w1_p,
            tc.tile_pool(name="w2", bufs=w2q_bufs) as w2_p,
            tc.tile_pool(name="b2", bufs=2) as b2_p,
            tc.tile_pool(name="xo", bufs=4) as xo_p,
            tc.tile_pool(name="stg", bufs=4) as stg_p,
            tc.tile_pool(name="ps", bufs=ps_bufs, space="PSUM") as ps_p,
        ):
            ones_sb = const_p.tile([P, P], f16, tag="ones")
            nc.any.memset(ones_sb[:], 1.0)

            b1_sb = {}
            for l, d in b1_d.items():
                b1_sb[l] = const_p.tile(list(d.shape), f32, tag=f"b1_{l}", name=f"b1_{l}")
                nc.scalar.dma_start(b1_sb[l][:], d[:])
            bhm_sb = const_p.tile([P, 1], f32, tag="bhm")
            nc.scalar.dma_start(bhm_sb[:], bhm_d[:])
            bhv_sb = const_p.tile([P, 1], f32, tag="bhv")
            nc.scalar.dma_start(bhv_sb[:], bhv_d[:])
            eps_sb = const_p.tile([P, NS], f32, tag="eps")
            nc.scalar.dma_start(eps_sb[:], eps_d[:])
            whm_sb = const_p.tile([P, O // P, P], f16, tag="whm")
            nc.scalar.dma_start(whm_sb[:], whm_d[:])
            whv_sb = const_p.tile([P, O // P, P], f16, tag="whv")
            nc.scalar.dma_start(whv_sb[:], whv_d[:])

            def all_gather(l, h):
                if collectives:
                    nc.gpsimd.collective_compute(
                        "AllGather", mybir.AluOpType.bypass, replica_groups=rg,
                        ins=[xown[l, h][:].opt()], outs=[xg[l, h][:].opt()],
                    )
                else:
                    for c in range(NC):
                        nc.sync.dma_start(xg[l, h][c * P:(c + 1) * P], xown[l, h][:])

            drain_ctr = [0]

            def drain(dst, psum):
                if drain_ctr[0] % 2 == 1:
                    nc.scalar.copy(dst, psum[:])
                else:
                    nc.vector.tensor_copy(dst, psum[:])
                drain_ctr[0] += 1

            def agg0():
                """Layer-0 aggregation: Mt=2 feature tiles, x0 streamed per n."""
                uT = big_p.tile([P, T // P, NS], f16, tag="uT", name="uT0")
                for n in range(ND):
                    psums = [ps_p.tile([P, 512], f32, tag="mm", name=f"a0ps{mi}")
                             for mi in range(2)]
                    for rb in range(4):
                        x0s = x_p.tile([P, 8, 2, T], f8, tag="xs")
                        nc.sync.dma_start(x0s[:], x0_d[rb])
                        for rr in range(2):
                            att = at_p.tile([P, 4, 2, 512], f8, tag="at")
                            nc.scalar.dma_start(att[:], at_d[n, rb * 2 + rr])
                            for kpl in range(4):
                                for mi in range(2):
                                    nc.tensor.matmul(
                                        psums[mi][:],
                                        lhsT=x0s[:, rr * 4 + kpl, :, mi * P:(mi + 1) * P],
                                        rhs=att[:, kpl],
                                        start=(rb == 0 and rr == 0 and kpl == 0),
                                        stop=(rb == 3 and rr == 1 and kpl == 3),
                                        perf_mode=DR,
                                    )
                    for mi in range(2):
                        drain(uT[:, mi, n * 512:(n + 1) * 512], psums[mi])
                return uT

            def agg(l):
                """Layers 1-3: 16 feature tiles; 8 psums cover one (n, h)."""
                uT = big_p.tile([P, H // P, NS], f16, tag="uT", name=f"uT{l}")
                for n in range(ND):
                    for h in range(2):
                        gh = xg[l, h]
                        psums = [ps_p.tile([P, 512], f32, tag="mm", name=f"agps{q}")
                                 for q in range(8)]
                        for r in range(NR):
                            xs = x_p.tile([P, 4, 2, 2, 512], f8, tag="xs")
                            nc.sync.dma_start(xs[:], gh[r * P:(r + 1) * P])
                            att = at_p.tile([P, 4, 2, 512], f8, tag="at")
                            nc.scalar.dma_start(att[:], at_d[n, r])
                            for kpl in range(4):
                                for fg in range(2):
                                    for mi in range(4):
                                        nc.tensor.matmul(
                                            psums[fg * 4 + mi][:],
                                            lhsT=xs[:, kpl, :, fg, mi * P:(mi + 1) * P],
                                            rhs=att[:, kpl],
                                            start=(r == 0 and kpl == 0),
                                            stop=(r == NR - 1 and kpl == 3),
                                            perf_mode=DR,
                                        )
                        for fg in range(2):
                            for mi in range(4):
                                drain(uT[:, (h * 2 + fg) * 4 + mi, n * 512:(n + 1) * 512],
                                      psums[fg * 4 + mi])
                return uT

            def lin1(l, uT, Kt, Mt):
                hT = big_p.tile([P, Mt, NS], f16, tag="hT", name=f"hT{l}")
                for mt in range(Mt):
                    ws = w1_p.tile([P, Kt, P], f16, tag="w1")
                    nc.scalar.dma_start(ws[:], w1_d[l][mt])
                    for n in range(ND):
                        p = ps_p.tile([P, 512], f32, tag="mm")
                        for k in range(Kt):
                            nc.tensor.matmul(
                                p[:],
                                lhsT=ws[:, k, :],
                                rhs=uT[:, k, n * 512:(n + 1) * 512],
                                start=(k == 0),
                                stop=(k == Kt - 1),
                            )
                        nc.scalar.activation(
                            hT[:, mt, n * 512:(n + 1) * 512], p[:],
                            AF.Relu, bias=b1_sb[l][:, mt:mt + 1],
                        )
                return hT

            def lin2(l, hT):
                """x_{l+1} = relu(h @ w2 + b2) node-major fp8 -> xown + AG."""
                Kt = H // P
                b2rep = b2_p.tile([P, H], f16, tag="b2rep")
                nc.scalar.dma_start(b2rep[:], b2r_d[l][:])
                for h in (0, 1):
                    w2q = []
                    for q in range(4):
                        wq = w2_p.tile([P, 4, HH], f16, tag="w2q", name=f"w2q{q}")
                        nc.scalar.dma_start(
                            wq[:], w2_d[l][:, q * 4:(q + 1) * 4, h * HH:(h + 1) * HH])
                        w2q.append(wq)
                    for kpl in range(NS // 256):
                        psums = [[ps_p.tile([P, 512], f32, tag="mm", name=f"l2ps{o}{ft}")
                                  for ft in range(2)] for o in range(2)]
                        for o in range(2):
                            for ft in range(2):
                                nc.tensor.matmul(
                                    psums[o][ft][:],
                                    lhsT=ones_sb[:],
                                    rhs=b2rep[:, h * HH + ft * 512:h * HH + (ft + 1) * 512],
                                    start=True, stop=False,
                                )
                        for kt in range(Kt):
                            wq = w2q[kt // 4]
                            for o in range(2):
                                j = kpl * 2 + o
                                for ft in range(2):
                                    nc.tensor.matmul(
                                        psums[o][ft][:],
                                        lhsT=hT[:, kt, j * P:(j + 1) * P],
                                        rhs=wq[:, kt % 4, ft * 512:(ft + 1) * 512],
                                        start=False, stop=(kt == Kt - 1),
                                    )
                        for o in range(2):
                            xo = xo_p.tile([P, 2, 512], f8, tag="xo")
                            for ft in range(2):
                                nc.scalar.activation(xo[:, ft, :], psums[o][ft][:], AF.Relu)
                            nc.sync.dma_start(xown[l + 1, h][:, kpl, o], xo[:])
                    all_gather(l + 1, h)

            # ---- layer 0 ----
            with nc.named_scope("l0_agg"):
                uT0 = agg0()
            with nc.named_scope("l0_lin1"):
                hT0 = lin1(0, uT0, T // P, H // P)
            with nc.named_scope("l0_lin2"):
                lin2(0, hT0)

            # ---- layers 1..3 ----
            hT3 = None
            for l in (1, 2, 3):
                with nc.named_scope(f"l{l}_agg"):
                    uT = agg(l)
                mt_out = (O if l == 3 else H) // P
                with nc.named_scope(f"l{l}_lin1"):
                    hT = lin1(l, uT, H // P, mt_out)
                if l < 3:
                    with nc.named_scope(f"l{l}_lin2"):
                        lin2(l, hT)
                else:
                    hT3 = hT

            # ---- fused heads ----
            with nc.named_scope("heads"):
                for n in range(ND):
                    nsl = slice(n * 512, (n + 1) * 512)
                    stages = {}
                    for key, W_sb, b_sb in (("m", whm_sb, bhm_sb), ("v", whv_sb, bhv_sb)):
                        p = ps_p.tile([P, 512], f32, tag="mm")
                        for k in range(O // P):
                            nc.tensor.matmul(
                                p[:],
                                lhsT=W_sb[:, k, :],
                                rhs=hT3[:, k, nsl],
                                start=(k == 0),
                                stop=(k == O // P - 1),
                            )
                        st = stg_p.tile([P, 512], f32, tag="stg", name=f"st{key}")
                        nc.scalar.activation(st[:], p[:], AF.Identity, bias=b_sb[:, 0:1])
                        stages[key] = st
                    stz = stg_p.tile([P, 512], f32, tag="stg", name="stz")
                    nc.vector.tensor_tensor(
                        stz[:], stages["v"][:], eps_sb[:, nsl], mybir.AluOpType.mult)
                    nc.vector.tensor_tensor(
                        stz[:], stz[:], stages["m"][:], mybir.AluOpType.add)
                    nc.sync.dma_start(mean_d[:, nsl], stages["m"][:])
                    nc.sync.dma_start(var_d[:, nsl], stages["v"][:])
                    nc.sync.dma_start(z_d[:, nsl], stz[:])

    nc.compile()
    return nc


def _tile_lhsT(w):
    """[K, M] fp16 -> [Mt, 128, Kt, 128]; slab [mt] is SBUF-ready [128p, Kt, 128m]."""
    K, M = w.shape
    Kt, Mt = K // P, M // P
    return np.ascontiguousarray(w.reshape(Kt, P, Mt, P).transpose(2, 1, 0, 3))


def _bias_t(b):
    """[M] fp32 -> [128, Mt] (partition = feature within tile)."""
    return np.ascontiguousarray(b.reshape(-1, P).T).astype(np.float32)


def _q8(x):
    return np.clip(x, -240.0, 240.0).astype(E4NP)


def prepare_inputs(inputs):
    """Host-side preprocessing: adjacency build + layout tiling. Returns in_maps."""
    f16 = np.float16
    eeg_nodes = np.asarray(inputs["eeg_nodes"], np.float32)
    eeg_idx = np.asarray(inputs["eeg_idx"])
    src = eeg_idx[0].astype(np.int64)
    dst = eeg_idx[1].astype(np.int64)

    counts = np.bincount(src * N + dst, minlength=N * N).reshape(N, N)
    AT = counts.astype(np.float32)
    AT[np.arange(N), np.arange(N)] += 1.0  # fold GIN's (1+eps)*x self-term, eps=0
    del counts

    # Scale layers 0-2 outputs by S=1/16 (exact power of 2, folded into w2/b2)
    # to keep activations in fp8/fp16 range; heads unscale via x S^-3.
    S = np.float32(1.0 / 16.0)
    c = [np.float32(1.0), S, S * S, S * S * S]  # cumulative scale of x_l input

    common = {}
    # x0 fp8: [rb, i, kpj, o, t] = x0[(rb*8+kpj)*256 + o*128 + i, t]
    common["x08"] = np.ascontiguousarray(
        _q8(eeg_nodes).reshape(4, 8, 2, P, T).transpose(0, 3, 1, 2, 4))
    for l in range(4):
        common[f"w1_{l}"] = _tile_lhsT(np.asarray(inputs[f"w1_{l}"], np.float32).astype(f16))
        common[f"b1_{l}"] = _bias_t(np.asarray(inputs[f"b1_{l}"], np.float32) * c[l])
    for l in range(3):
        w2s = (np.asarray(inputs[f"w2_{l}"], np.float32) * S).astype(f16)
        # K-major: [i, kt, m] = w2[kt*128 + i, m]
        common[f"w2k_{l}"] = np.ascontiguousarray(
            w2s.reshape(H // P, P, H).transpose(1, 0, 2))
        b2s = (np.asarray(inputs[f"b2_{l}"], np.float32) * c[l + 1]) / np.float32(P)
        common[f"b2r_{l}"] = np.ascontiguousarray(
            np.broadcast_to(b2s.astype(f16), (P, H)))

    # fused heads: mean = h3 @ (w2_3 @ wm) + (b2_3 @ wm + bm); h3 carries c[3]
    w2_3 = np.asarray(inputs["w2_3"], np.float32)
    b2_3 = np.asarray(inputs["b2_3"], np.float32)
    wm = np.asarray(inputs["wm"], np.float32)
    wv = np.asarray(inputs["wv"], np.float32)
    W2m = ((w2_3 @ wm) / c[3]).astype(f16)
    W2v = ((w2_3 @ wv) / c[3]).astype(f16)
    common["whm"] = _tile_lhsT(W2m)[0]
    common["whv"] = _tile_lhsT(W2v)[0]
    common["bhm"] = (b2_3 @ wm + np.asarray(inputs["bm"], np.float32)).reshape(P, 1).astype(np.float32)
    common["bhv"] = (b2_3 @ wv + np.asarray(inputs["bv"], np.float32)).reshape(P, 1).astype(np.float32)

    eps = np.asarray(inputs["eps"], np.float32)
    in_maps = []
    for cc in range(NC):
        m = dict(common)
        blk = AT[:, cc * NS:(cc + 1) * NS]
        # [n, r, i, kpl, o, f] = AT[(r*4+kpl)*256 + o*128 + i, n*512 + f]
        m["at8"] = np.ascontiguousarray(
            _q8(blk.reshape(NR, 4, 2, P, ND, 512).transpose(4, 0, 3, 1, 2, 5)))
        m["epst"] = np.ascontiguousarray(eps[cc * NS:(cc + 1) * NS, :].T)
        in_maps.append(m)
    return in_maps


def get_program():
    if "nc" not in _PROGRAM_CACHE:
        _PROGRAM_CACHE["nc"] = _build_program()
    return _PROGRAM_CACHE["nc"]


def assemble_outputs(results):
    z = np.empty((N, L), np.float32)
    mean = np.empty((N, L), np.float32)
    var = np.empty((N, L), np.float32)
    for c in range(NC):
        z[c * NS:(c + 1) * NS] = results[c]["zt"].T
        mean[c * NS:(c + 1) * NS] = results[c]["meant"].T
        var[c * NS:(c + 1) * NS] = results[c]["vart"].T
    return z, mean, var


def kernel(**inputs):
    from concourse.bass_utils import run_bass_kernel_spmd

    nc = get_program()
    in_maps = prepare_inputs(inputs)
    res = run_bass_kernel_spmd(nc, in_maps, core_ids=list(range(NC)))
    return assemble_outputs(res.results)
